# revision 2
# baseline (speedup 1.0000x reference)
"""LocalPatchAttention Trainium2 kernel, v2.

Data-parallel over batch B=8 across 8 NeuronCores (one image per core).

Host-side folds (per image, all channel math exact in f32):
  - LayerNorm mean-centering of q is linear, so it folds into the logits
    matrix: At = 8*(A0 - colmean(A0)), A0 = scale*(g*qW^T)@K^T.  The 8
    compensates r = 1/sqrt(sum q^2) vs 1/sqrt(mean q^2) (sqrt(64)=8).
  - The mu^2 term of the variance is dropped (mu ~ N(0,1/64): ~0.8% var
    error -> ~0.4% on r) and eps=1e-5 is negligible vs E[q^2]~1.
  - Same folds on the V path (sqrt(128) scale, vbp added via Act copy).

Per-core pipeline (4 blocks x 32 pairs; pair = 2 image rows packed as
[128 part = 2rows x 64ch, 256 px]):
  A-phase: DMA qin group [128,512]; qsq=qin^2 (Pool, bf16); SSexp =
    bexp^T @ qsq (one matmul: per-pixel channel-sums broadcast to all
    128 partitions); sq=sqrt(SSexp) (Act); r=1/sq (DVE).  All Act ops
    in A-phases use the sqrt table; sigmoids batch in B-phases, so the
    1283ns activation-table reload happens 8x per image, not per pair.
  B-phase: qs = qin*r (DVE, bf16); logits = At^T @ qs (2 matmuls into
    [128v, 512]); sig = Sigmoid(lg + cb) (Act, bf16); srow = sig*V
    (fp8, DVE/Pool alternating); conv = 12 fp8 DoubleRow matmuls per
    pair (2 taps contracted per pass, 0.5 cyc/col); residual+conv-bias
    via one scalar_tensor_tensor (DVE); out DMA.
"""

import numpy as np
import ml_dtypes

import concourse.bass as bass
import concourse.bacc as bacc
import concourse.tile as tile
from concourse import mybir
from concourse.bass_utils import run_bass_kernel_spmd

F32 = mybir.dt.float32
BF16 = mybir.dt.bfloat16
FP8 = mybir.dt.float8e4
AF = mybir.ActivationFunctionType
ALU = mybir.AluOpType
DR = mybir.MatmulPerfMode.DoubleRow
NPBF16 = ml_dtypes.bfloat16
NPFP8 = ml_dtypes.float8_e4m3

_CACHE = {}

NG = 64          # qin groups (2 pairs each)
BLK = 32         # groups per phase-block
NBLK = NG // BLK
LAG = 16         # pairs of conv delay, spills PE work into A-phases


def _build_nc():
    nc = bacc.Bacc()
    q_d = nc.declare_dram_parameter("q2", [128, 32768], BF16, isOutput=False)
    v_d = nc.declare_dram_parameter("v2", [128, 4096], BF16, isOutput=False)
    A_d = nc.declare_dram_parameter("Abig", [128, 256], BF16, isOutput=False)
    bexp_d = nc.declare_dram_parameter("bexp", [128, 128], BF16, isOutput=False)
    cw8_d = nc.declare_dram_parameter("cw8", [128, 1536], FP8, isOutput=False)
    cbs_d = nc.declare_dram_parameter("cbs", [128, 1], F32, isOutput=False)
    vN_d = nc.declare_dram_parameter("vN", [128, 128], BF16, isOutput=False)
    vbp_d = nc.declare_dram_parameter("vbp", [128, 1], F32, isOutput=False)
    onesf_d = nc.declare_dram_parameter("onesf", [128, 128], BF16, isOutput=False)
    cb8_d = nc.declare_dram_parameter("cb8", [1, 128], FP8, isOutput=False)
    out_d = nc.declare_dram_parameter("out2", [64, 65536], FP8, isOutput=True)

    with nc.allow_low_precision(reason="bf16/fp8 validated vs reference (rel 7e-3)"), \
         tile.TileContext(nc) as tc, \
         tc.tile_pool(name="const", bufs=1) as cpool, \
         tc.tile_pool(name="vV", bufs=1) as vV_pool, \
         tc.tile_pool(name="qin", bufs=52) as qin_pool, \
         tc.tile_pool(name="qsq", bufs=4) as qsq_pool, \
         tc.tile_pool(name="sqp", bufs=3) as sq_pool, \
         tc.tile_pool(name="rp", bufs=26) as r_pool, \
         tc.tile_pool(name="qsp", bufs=4) as qs_pool, \
         tc.tile_pool(name="sigp", bufs=4) as sig_pool, \
         tc.tile_pool(name="srowp", bufs=22) as srow_pool, \
         tc.tile_pool(name="otp", bufs=5) as ot_pool, \
         tc.tile_pool(name="tokp", bufs=2) as tok_pool, \
         tc.tile_pool(name="ps_sx", bufs=2, space="PSUM") as ps_sx:

        def const_tile(shape, dtype, tag, src):
            t = cpool.tile(shape, dtype, tag=tag)
            nc.sync.dma_start(out=t, in_=src[:, :])
            return t

        A_sb = const_tile([128, 256], BF16, "A", A_d)
        bexp_sb = const_tile([128, 128], BF16, "bexp", bexp_d)
        cw8_sb = const_tile([128, 1536], FP8, "cw8", cw8_d)
        cbs_sb = const_tile([128, 1], F32, "cbs", cbs_d)
        vN_sb = const_tile([128, 128], BF16, "vN", vN_d)
        vbp_sb = const_tile([128, 1], F32, "vbp", vbp_d)
        onesf_sb = const_tile([128, 128], BF16, "onesf", onesf_d)
        cb8_sb = cpool.tile([1, 128], FP8, tag="cb8")
        nc.sync.dma_start(out=cb8_sb, in_=cb8_d[:, :])
        ones8 = cpool.tile([1, 1024], FP8, tag="ones8")
        nc.vector.memset(ones8, 1.0)
        onescol = cpool.tile([128, 1], BF16, tag="onescol")
        nc.vector.memset(onescol, 1.0)

        # ---------------- V path ----------------
        vw_ctx = tc.tile_pool(name="vwork", bufs=2)
        vw_pool = vw_ctx.__enter__()
        psv_ctx = tc.tile_pool(name="ps_v", bufs=2, space="PSUM")
        ps_v = psv_ctx.__enter__()
        V_sb = vV_pool.tile([128, 4096], BF16, tag="V")
        for c in range(8):
            sl = slice(c * 512, (c + 1) * 512)
            vraw = vw_pool.tile([128, 512], BF16, tag="vraw")
            nc.sync.dma_start(out=vraw, in_=v_d[:, sl])
            vb16 = vraw
            vsq = vw_pool.tile([128, 512], BF16, tag="vsq")
            nc.gpsimd.tensor_mul(vsq, vraw, vraw)
            ssv = ps_v.tile([128, 512], F32, tag="vps")
            nc.tensor.matmul(ssv, onesf_sb, vsq, start=True, stop=True)
            sqv = vw_pool.tile([128, 512], BF16, tag="sqv")
            nc.scalar.activation(sqv, ssv, AF.Sqrt)
            rv = vw_pool.tile([128, 512], BF16, tag="rv")
            nc.vector.reciprocal(rv, sqv)
            vmm = ps_v.tile([128, 512], F32, tag="vps")
            nc.tensor.matmul(vmm, vN_sb, vb16, start=True, stop=True)
            vt = vw_pool.tile([128, 512], F32, tag="vt")
            nc.vector.tensor_mul(vt, vmm, rv)
            nc.gpsimd.tensor_tensor(
                V_sb[:, sl], vt,
                vbp_sb[:, 0:1].broadcast_to([128, 512]), ALU.add)
        psv_ctx.__exit__(None, None, None)
        vw_ctx.__exit__(None, None, None)
        pslg_ctx = tc.tile_pool(name="ps_lg", bufs=4, space="PSUM")
        ps_lg = pslg_ctx.__enter__()
        pscv_ctx = tc.tile_pool(name="ps_cv", bufs=2, space="PSUM")
        ps_cv = pscv_ctx.__enter__()

        # ---------------- main loop ----------------
        qins = {}
        rs = {}
        srows = {}
        otg = {}
        # Act-phase gating tokens: the scale operand of sqrt (sigmoid) ops is
        # a [128,1] ones tile derived from the previous sigmoid (sqrt) phase's
        # last output, forcing strict sqrt/sigmoid phase separation on the
        # Activation engine so the 1283ns act-table reload happens only at
        # phase boundaries.
        gate = {"a": None, "b": None}

        def make_gate(src_col):
            # Identity(in*0 + 1) = 1.0; Identity lives in every act table so
            # this adds no table switch, and the data dep does the gating.
            tok = tok_pool.tile([128, 1], F32, tag="tok")
            nc.scalar.activation(tok, src_col, AF.Identity, scale=0.0,
                                 bias=1.0)
            return tok

        def prefetch(g):
            qin = qin_pool.tile([128, 512], BF16, tag="qin")
            nc.sync.dma_start(out=qin, in_=q_d[:, g * 512:(g + 1) * 512])
            qins[g] = qin

        sqd = {}

        def phase_a(g):
            qin = qins[g]
            qsq = qsq_pool.tile([128, 512], BF16, tag="qsq")
            nc.gpsimd.tensor_mul(qsq, qin, qin)
            ssx = ps_sx.tile([128, 512], F32, tag="sx")
            nc.tensor.matmul(ssx, bexp_sb, qsq, start=True, stop=True)
            if g % 2 == 0:
                sq_t = sq_pool.tile([128, 1024], BF16, tag="sq")
                sqd[0] = sq_t
            sq = sqd[0]
            scl = 1.0 if gate["b"] is None else gate["b"][:, 0:1]
            nc.scalar.activation(sq[:, (g % 2) * 512:(g % 2 + 1) * 512], ssx,
                                 AF.Sqrt, scale=scl)
            if g % 2 == 1:
                r = r_pool.tile([128, 1024], BF16, tag="r")
                nc.vector.reciprocal(r, sq)
                rs[g - 1] = r[:, 0:512]
                rs[g] = r[:, 512:1024]
            return rs.get(g)

        def make_srow(p, sig):
            g = p // 2
            vb = V_sb[:, g * 64:(g + 1) * 64]
            vb_ap = vb.rearrange("p c -> p c ()").broadcast_to([128, 64, 4])
            srow = srow_pool.tile([128, 512], FP8, tag="srow")
            for rr in (0, 1):
                eng = nc.gpsimd
                eng.tensor_tensor(
                    srow[:, rr * 256:(rr + 1) * 256].rearrange(
                        "p (c f) -> p c f", f=4),
                    sig[:, rr * 256:(rr + 1) * 256].rearrange(
                        "p (c f) -> p c f", f=4),
                    vb_ap, ALU.mult)
            srows[p] = srow

        def conv_pair(p):
            cv = ps_cv.tile([64, 512], F32, tag="cv")
            # the first MM (h=0, dx=1, full width) carries start=True; its
            # 2KB pending-zero covers the whole bank, so the h=1 half (all
            # start=False) accumulates onto hw-zeroed bytes.
            nmm = [0]

            def mm(out_ap, wt, rt_ap):
                nmm[0] += 1
                nc.tensor.matmul(out_ap, wt, rt_ap, start=(nmm[0] == 1),
                                 stop=(nmm[0] == total[0]),
                                 skip_group_check=True, perf_mode=DR)

            total = [0]
            for h in (0, 1):
                total[0] += 3
                t = p - 1 if h == 0 else p + 1
                if 0 <= t <= 127 and t in srows:
                    total[0] += 3
            for h in (0, 1):
                base = h * 256
                mms = []
                for dx in (1, 0, 2):
                    mms.append((0, p, dx))
                    t = p - 1 if h == 0 else p + 1
                    if 0 <= t <= 127 and t in srows:
                        mms.append((1, t, dx))
                for e, t, dx in mms:
                    blk = (h * 6 + dx * 2 + e) * 128
                    wt = cw8_sb[:, blk:blk + 128].rearrange(
                        "p (two m) -> p two m", two=2)
                    rt = srows[t].rearrange("p (r n) -> p r n", r=2)
                    if dx == 1:
                        mm(cv[:, base:base + 256], wt, rt)
                    elif dx == 0:
                        mm(cv[:, base + 1:base + 256], wt, rt[:, :, 0:255])
                    else:
                        mm(cv[:, base:base + 255], wt, rt[:, :, 1:256])
            # conv+bias result -> bf16 out tile spanning 2 pairs; the q
            # residual is added on the host (exact f32 there).
            t2, half = p // 2, p % 2
            if half == 0:
                ot = ot_pool.tile([64, 1024], FP8, tag="ot")
                otg[t2] = ot
            ot = otg[t2]
            nc.vector.tensor_copy(ot[:, half * 512:(half + 1) * 512], cv)
            if half == 1:
                nc.sync.dma_start(out=out_d[:, t2 * 1024:(t2 + 1) * 1024], in_=ot)
                del otg[t2]
            for t in list(srows):
                if t < p - 1:
                    del srows[t]

        next_conv = [0]

        def drive_conv(upto):
            while next_conv[0] <= min(upto, 127):
                conv_pair(next_conv[0])
                next_conv[0] += 1

        def phase_b(g):
            qin = qins.pop(g)
            qs = qs_pool.tile([128, 512], BF16, tag="qs")
            nc.gpsimd.tensor_mul(qs, qin, rs.pop(g))
            last_sig = None
            for h2 in (0, 1):
                p = 2 * g + h2
                lg = ps_lg.tile([128, 512], F32, tag="lg")
                csl = slice(h2 * 256, (h2 + 1) * 256)
                nc.tensor.matmul(lg[:, 0:256], A_sb[:, 0:128], qs[:, csl],
                                 start=True, stop=True)
                nc.tensor.matmul(lg[:, 256:512], A_sb[:, 128:256], qs[:, csl],
                                 start=True, stop=True)
                sig = sig_pool.tile([128, 512], BF16, tag="sig")
                nc.scalar.activation(sig, lg, AF.Sigmoid, bias=cbs_sb[:, 0:1],
                                     scale=gate["a"][:, 0:1])
                last_sig = sig
                make_srow(p, sig)
                lag = LAG if p < 112 else max(1, LAG - (p - 112))
                drive_conv(p - lag)
            return last_sig

        BLOCKS = [24, 40]
        starts = [sum(BLOCKS[:i]) for i in range(len(BLOCKS))]
        for g in range(BLOCKS[0]):
            prefetch(g)
        fetched = BLOCKS[0]
        for bi, bsz in enumerate(BLOCKS):
            g0 = starts[bi]
            last_r = None
            for g in range(g0, g0 + bsz):
                last_r = phase_a(g)
            gate["a"] = make_gate(last_r[:, 0:1])
            last_sig = None
            nxt = BLOCKS[bi + 1] if bi + 1 < len(BLOCKS) else 0
            for j, g in enumerate(range(g0, g0 + bsz)):
                # prefetch the next block's groups spread over this B phase
                while fetched < min(NG, g0 + bsz + nxt) and \
                        (j + 1) * nxt >= (fetched - g0 - bsz + 1) * bsz:
                    prefetch(fetched)
                    fetched += 1
                last_sig = phase_b(g)
            gate["b"] = make_gate(last_sig[:, 0:1])
        drive_conv(127)
        pscv_ctx.__exit__(None, None, None)
        pslg_ctx.__exit__(None, None, None)

    nc.finalize()
    return nc


def _fold_weights(qW, qb, vW, vb, K, qn_g, qn_b, vn_g, vn_b, cW, cb):
    f = np.float32
    qW, qb, vW, vb, K = f(qW), f(qb), f(vW), f(vb), f(K)
    qn_g, qn_b, vn_g, vn_b, cW, cb = f(qn_g), f(qn_b), f(vn_g), f(vn_b), f(cW), f(cb)
    scale = np.float32(64.0 ** -0.5)
    qWf = qn_g[:, None] * qW.T                      # [64k, 64co]
    bprime = qb + qW @ qn_b
    A0 = scale * (qWf @ K.T)                        # [64, 128]
    cbs = scale * (K @ bprime)                      # [128]
    At = 8.0 * (A0 - A0.mean(axis=0, keepdims=True))
    # K=128 zero-padded lhsT blocks: block0 contracts rows 0-63 (row y of the
    # pair), block1 rows 64-127 (row y+1).  Avoids K=64 matmuls at partition
    # base 64, which crash at runtime (bad tile_position (64,0)).
    Abig = np.zeros((128, 256), np.float32)
    Abig[0:64, 0:128] = At
    Abig[64:128, 128:256] = At

    bexp = np.zeros((128, 128), np.float32)
    bexp[0:64, 0:64] = 1.0
    bexp[64:128, 64:128] = 1.0

    vWf = vn_g[:, None] * vW.T                      # [128, 128]
    vbp = vb + vW @ vn_b
    vN = np.float32(np.sqrt(128.0)) * (vWf - vWf.mean(axis=0, keepdims=True))

    cw8 = np.zeros((128, 12, 2, 64), np.float32)
    for h in (0, 1):
        for dx in (0, 1, 2):
            for e in (0, 1):
                blk = h * 6 + dx * 2 + e
                if h == 0 and e == 0:
                    cw8[:, blk, 0, :] = cW[:, :, 1, dx].T
                    cw8[:, blk, 1, :] = cW[:, :, 2, dx].T
                elif h == 0 and e == 1:
                    cw8[:, blk, 1, :] = cW[:, :, 0, dx].T
                elif h == 1 and e == 0:
                    cw8[:, blk, 0, :] = cW[:, :, 0, dx].T
                    cw8[:, blk, 1, :] = cW[:, :, 1, dx].T
                else:
                    cw8[:, blk, 0, :] = cW[:, :, 2, dx].T

    onesf = np.ones((128, 128), np.float32)
    # conv bias K=1 matmul lhsT [1, 64]
    cb8 = np.zeros((1, 2, 64), np.float32)
    cb8[0, 0, :] = cb

    return {
        "Abig": np.ascontiguousarray(Abig.astype(NPBF16)),
        "bexp": np.ascontiguousarray(bexp.astype(NPBF16)),
        "cw8": np.ascontiguousarray(cw8.reshape(128, 1536).astype(NPFP8)),
        "cbs": np.ascontiguousarray(cbs.reshape(128, 1)),
        "vN": np.ascontiguousarray(vN.astype(NPBF16)),
        "vbp": np.ascontiguousarray(vbp.reshape(128, 1)),
        "onesf": np.ascontiguousarray(onesf.astype(NPBF16)),
        "cb8": np.ascontiguousarray(cb8.reshape(1, 128).astype(NPFP8)),
    }


def _pack_q(qi):
    # q [64, 256, 256] -> [128, 32768]: (c, 2j+r, x) -> [r*64+c, j*256+x]
    return np.ascontiguousarray(
        np.float32(qi).reshape(64, 128, 2, 256).transpose(2, 0, 1, 3)
        .reshape(128, 32768).astype(NPBF16))


def _unpack_out(o2):
    # conv result is already row-major [64ch, 256*256]
    return np.asarray(o2, np.float32).reshape(64, 256, 256)


def _run(in_maps, **kw):
    if "nc" not in _CACHE:
        _CACHE["nc"] = _build_nc()
    return run_bass_kernel_spmd(_CACHE["nc"], in_maps, list(range(8)), **kw)


def kernel(q, v, qW, qb, vW, vb, K, qn_g, qn_b, vn_g, vn_b, cW, cb):
    base = _fold_weights(qW, qb, vW, vb, K, qn_g, qn_b, vn_g, vn_b, cW, cb)
    in_maps = []
    for i in range(8):
        m = dict(base)
        m["q2"] = _pack_q(q[i])
        m["v2"] = np.ascontiguousarray(np.float32(v[i]).reshape(128, 4096).astype(NPBF16))
        in_maps.append(m)
    res = _run(in_maps)
    qcb = np.float32(q) + np.float32(cb)[None, :, None, None]
    outs = [qcb[i] + _unpack_out(res.results[i]["out2"]) for i in range(8)]
    return np.stack(outs)


# revision 3
# speedup vs baseline: 1.0083x; 1.0083x over previous
"""LocalPatchAttention Trainium2 kernel, v2.

Data-parallel over batch B=8 across 8 NeuronCores (one image per core).

Host-side folds (per image, all channel math exact in f32):
  - LayerNorm mean-centering of q is linear, so it folds into the logits
    matrix: At = 8*(A0 - colmean(A0)), A0 = scale*(g*qW^T)@K^T.  The 8
    compensates r = 1/sqrt(sum q^2) vs 1/sqrt(mean q^2) (sqrt(64)=8).
  - The mu^2 term of the variance is dropped (mu ~ N(0,1/64): ~0.8% var
    error -> ~0.4% on r) and eps=1e-5 is negligible vs E[q^2]~1.
  - Same folds on the V path (sqrt(128) scale, vbp added via Act copy).

Per-core pipeline (4 blocks x 32 pairs; pair = 2 image rows packed as
[128 part = 2rows x 64ch, 256 px]):
  A-phase: DMA qin group [128,512]; qsq=qin^2 (Pool, bf16); SSexp =
    bexp^T @ qsq (one matmul: per-pixel channel-sums broadcast to all
    128 partitions); sq=sqrt(SSexp) (Act); r=1/sq (DVE).  All Act ops
    in A-phases use the sqrt table; sigmoids batch in B-phases, so the
    1283ns activation-table reload happens 8x per image, not per pair.
  B-phase: qs = qin*r (DVE, bf16); logits = At^T @ qs (2 matmuls into
    [128v, 512]); sig = Sigmoid(lg + cb) (Act, bf16); srow = sig*V
    (fp8, DVE/Pool alternating); conv = 12 fp8 DoubleRow matmuls per
    pair (2 taps contracted per pass, 0.5 cyc/col); residual+conv-bias
    via one scalar_tensor_tensor (DVE); out DMA.
"""

import numpy as np
import ml_dtypes

import concourse.bass as bass
import concourse.bacc as bacc
import concourse.tile as tile
from concourse import mybir
from concourse.bass_utils import run_bass_kernel_spmd

F32 = mybir.dt.float32
BF16 = mybir.dt.bfloat16
FP8 = mybir.dt.float8e4
AF = mybir.ActivationFunctionType
ALU = mybir.AluOpType
DR = mybir.MatmulPerfMode.DoubleRow
NPBF16 = ml_dtypes.bfloat16
NPFP8 = ml_dtypes.float8_e4m3

_CACHE = {}

NG = 64          # qin groups (2 pairs each)
BLK = 32         # groups per phase-block
NBLK = NG // BLK
LAG = 24         # pairs of conv delay, spills PE work into A-phases


def _build_nc():
    nc = bacc.Bacc()
    q_d = nc.declare_dram_parameter("q2", [128, 32768], BF16, isOutput=False)
    v_d = nc.declare_dram_parameter("v2", [128, 4096], BF16, isOutput=False)
    A_d = nc.declare_dram_parameter("Abig", [128, 256], BF16, isOutput=False)
    bexp_d = nc.declare_dram_parameter("bexp", [128, 128], BF16, isOutput=False)
    cw8_d = nc.declare_dram_parameter("cw8", [128, 1536], FP8, isOutput=False)
    cbs_d = nc.declare_dram_parameter("cbs", [128, 1], F32, isOutput=False)
    vN_d = nc.declare_dram_parameter("vN", [128, 128], BF16, isOutput=False)
    vbp_d = nc.declare_dram_parameter("vbp", [128, 1], F32, isOutput=False)
    onesf_d = nc.declare_dram_parameter("onesf", [128, 128], BF16, isOutput=False)
    cb8_d = nc.declare_dram_parameter("cb8", [1, 128], FP8, isOutput=False)
    out_d = nc.declare_dram_parameter("out2", [64, 65536], FP8, isOutput=True)

    with nc.allow_low_precision(reason="bf16/fp8 validated vs reference (rel 7e-3)"), \
         tile.TileContext(nc) as tc, \
         tc.tile_pool(name="const", bufs=1) as cpool, \
         tc.tile_pool(name="vV", bufs=1) as vV_pool, \
         tc.tile_pool(name="qin", bufs=52) as qin_pool, \
         tc.tile_pool(name="qsq", bufs=4) as qsq_pool, \
         tc.tile_pool(name="sqp", bufs=3) as sq_pool, \
         tc.tile_pool(name="rp", bufs=26) as r_pool, \
         tc.tile_pool(name="qsp", bufs=4) as qs_pool, \
         tc.tile_pool(name="sigp", bufs=4) as sig_pool, \
         tc.tile_pool(name="srowp", bufs=30) as srow_pool, \
         tc.tile_pool(name="otp", bufs=5) as ot_pool, \
         tc.tile_pool(name="tokp", bufs=2) as tok_pool, \
         tc.tile_pool(name="ps_sx", bufs=2, space="PSUM") as ps_sx:

        def const_tile(shape, dtype, tag, src):
            t = cpool.tile(shape, dtype, tag=tag)
            nc.sync.dma_start(out=t, in_=src[:, :])
            return t

        A_sb = const_tile([128, 256], BF16, "A", A_d)
        bexp_sb = const_tile([128, 128], BF16, "bexp", bexp_d)
        cw8_sb = const_tile([128, 1536], FP8, "cw8", cw8_d)
        cbs_sb = const_tile([128, 1], F32, "cbs", cbs_d)
        vN_sb = const_tile([128, 128], BF16, "vN", vN_d)
        vbp_sb = const_tile([128, 1], F32, "vbp", vbp_d)
        onesf_sb = const_tile([128, 128], BF16, "onesf", onesf_d)
        cb8_sb = cpool.tile([1, 128], FP8, tag="cb8")
        nc.sync.dma_start(out=cb8_sb, in_=cb8_d[:, :])
        ones8 = cpool.tile([1, 1024], FP8, tag="ones8")
        nc.vector.memset(ones8, 1.0)
        onescol = cpool.tile([128, 1], BF16, tag="onescol")
        nc.vector.memset(onescol, 1.0)

        # ---------------- V path ----------------
        vw_ctx = tc.tile_pool(name="vwork", bufs=2)
        vw_pool = vw_ctx.__enter__()
        psv_ctx = tc.tile_pool(name="ps_v", bufs=2, space="PSUM")
        ps_v = psv_ctx.__enter__()
        V_sb = vV_pool.tile([128, 4096], BF16, tag="V")
        for c in range(8):
            sl = slice(c * 512, (c + 1) * 512)
            vraw = vw_pool.tile([128, 512], BF16, tag="vraw")
            nc.sync.dma_start(out=vraw, in_=v_d[:, sl])
            vb16 = vraw
            vsq = vw_pool.tile([128, 512], BF16, tag="vsq")
            nc.gpsimd.tensor_mul(vsq, vraw, vraw)
            ssv = ps_v.tile([128, 512], F32, tag="vps")
            nc.tensor.matmul(ssv, onesf_sb, vsq, start=True, stop=True)
            sqv = vw_pool.tile([128, 512], BF16, tag="sqv")
            nc.scalar.activation(sqv, ssv, AF.Sqrt)
            rv = vw_pool.tile([128, 512], BF16, tag="rv")
            nc.vector.reciprocal(rv, sqv)
            vmm = ps_v.tile([128, 512], F32, tag="vps")
            nc.tensor.matmul(vmm, vN_sb, vb16, start=True, stop=True)
            vt = vw_pool.tile([128, 512], F32, tag="vt")
            nc.vector.tensor_mul(vt, vmm, rv)
            nc.gpsimd.tensor_tensor(
                V_sb[:, sl], vt,
                vbp_sb[:, 0:1].broadcast_to([128, 512]), ALU.add)
        psv_ctx.__exit__(None, None, None)
        vw_ctx.__exit__(None, None, None)
        pslg_ctx = tc.tile_pool(name="ps_lg", bufs=3, space="PSUM")
        ps_lg = pslg_ctx.__enter__()
        pscv_ctx = tc.tile_pool(name="ps_cv", bufs=3, space="PSUM")
        ps_cv = pscv_ctx.__enter__()

        # ---------------- main loop ----------------
        qins = {}
        rs = {}
        srows = {}
        otg = {}
        # Act-phase gating tokens: the scale operand of sqrt (sigmoid) ops is
        # a [128,1] ones tile derived from the previous sigmoid (sqrt) phase's
        # last output, forcing strict sqrt/sigmoid phase separation on the
        # Activation engine so the 1283ns act-table reload happens only at
        # phase boundaries.
        gate = {"a": None, "b": None}

        def make_gate(src_col):
            # Identity(in*0 + 1) = 1.0; Identity lives in every act table so
            # this adds no table switch, and the data dep does the gating.
            tok = tok_pool.tile([128, 1], F32, tag="tok")
            nc.scalar.activation(tok, src_col, AF.Identity, scale=0.0,
                                 bias=1.0)
            return tok

        def prefetch(g):
            qin = qin_pool.tile([128, 512], BF16, tag="qin")
            nc.sync.dma_start(out=qin, in_=q_d[:, g * 512:(g + 1) * 512])
            qins[g] = qin

        sqd = {}

        def phase_a(g):
            qin = qins[g]
            qsq = qsq_pool.tile([128, 512], BF16, tag="qsq")
            nc.gpsimd.tensor_mul(qsq, qin, qin)
            ssx = ps_sx.tile([128, 512], F32, tag="sx")
            nc.tensor.matmul(ssx, bexp_sb, qsq, start=True, stop=True)
            if g % 2 == 0:
                sq_t = sq_pool.tile([128, 1024], BF16, tag="sq")
                sqd[0] = sq_t
            sq = sqd[0]
            scl = 1.0 if gate["b"] is None else gate["b"][:, 0:1]
            nc.scalar.activation(sq[:, (g % 2) * 512:(g % 2 + 1) * 512], ssx,
                                 AF.Sqrt, scale=scl)
            if g % 2 == 1:
                r = r_pool.tile([128, 1024], BF16, tag="r")
                nc.vector.reciprocal(r, sq)
                rs[g - 1] = r[:, 0:512]
                rs[g] = r[:, 512:1024]
            return rs.get(g)

        def make_srow(p, sig):
            g = p // 2
            vb = V_sb[:, g * 64:(g + 1) * 64]
            vb_ap = vb.rearrange("p c -> p c ()").broadcast_to([128, 64, 4])
            srow = srow_pool.tile([128, 512], FP8, tag="srow")
            for rr in (0, 1):
                eng = nc.gpsimd
                eng.tensor_tensor(
                    srow[:, rr * 256:(rr + 1) * 256].rearrange(
                        "p (c f) -> p c f", f=4),
                    sig[:, rr * 256:(rr + 1) * 256].rearrange(
                        "p (c f) -> p c f", f=4),
                    vb_ap, ALU.mult)
            srows[p] = srow

        def conv_pair(p):
            cv = ps_cv.tile([64, 512], F32, tag="cv")
            # the first MM (h=0, dx=1, full width) carries start=True; its
            # 2KB pending-zero covers the whole bank, so the h=1 half (all
            # start=False) accumulates onto hw-zeroed bytes.
            nmm = [0]

            def mm(out_ap, wt, rt_ap):
                nmm[0] += 1
                nc.tensor.matmul(out_ap, wt, rt_ap, start=(nmm[0] == 1),
                                 stop=(nmm[0] == total[0]),
                                 skip_group_check=True, perf_mode=DR)

            total = [0]
            for h in (0, 1):
                total[0] += 3
                t = p - 1 if h == 0 else p + 1
                if 0 <= t <= 127 and t in srows:
                    total[0] += 3
            for h in (0, 1):
                base = h * 256
                mms = []
                for dx in (1, 0, 2):
                    mms.append((0, p, dx))
                    t = p - 1 if h == 0 else p + 1
                    if 0 <= t <= 127 and t in srows:
                        mms.append((1, t, dx))
                for e, t, dx in mms:
                    blk = (h * 6 + dx * 2 + e) * 128
                    wt = cw8_sb[:, blk:blk + 128].rearrange(
                        "p (two m) -> p two m", two=2)
                    rt = srows[t].rearrange("p (r n) -> p r n", r=2)
                    if dx == 1:
                        mm(cv[:, base:base + 256], wt, rt)
                    elif dx == 0:
                        mm(cv[:, base + 1:base + 256], wt, rt[:, :, 0:255])
                    else:
                        mm(cv[:, base:base + 255], wt, rt[:, :, 1:256])
            # conv+bias result -> bf16 out tile spanning 2 pairs; the q
            # residual is added on the host (exact f32 there).
            t2, half = p // 2, p % 2
            if half == 0:
                ot = ot_pool.tile([64, 1024], FP8, tag="ot")
                otg[t2] = ot
            ot = otg[t2]
            nc.vector.tensor_copy(ot[:, half * 512:(half + 1) * 512], cv)
            if half == 1:
                nc.sync.dma_start(out=out_d[:, t2 * 1024:(t2 + 1) * 1024], in_=ot)
                del otg[t2]
            for t in list(srows):
                if t < p - 1:
                    del srows[t]

        next_conv = [0]

        def drive_conv(upto):
            while next_conv[0] <= min(upto, 127):
                conv_pair(next_conv[0])
                next_conv[0] += 1

        def phase_b(g):
            qin = qins.pop(g)
            qs = qs_pool.tile([128, 512], BF16, tag="qs")
            nc.gpsimd.tensor_mul(qs, qin, rs.pop(g))
            last_sig = None
            for h2 in (0, 1):
                p = 2 * g + h2
                lg = ps_lg.tile([128, 512], F32, tag="lg")
                csl = slice(h2 * 256, (h2 + 1) * 256)
                nc.tensor.matmul(lg[:, 0:256], A_sb[:, 0:128], qs[:, csl],
                                 start=True, stop=True)
                nc.tensor.matmul(lg[:, 256:512], A_sb[:, 128:256], qs[:, csl],
                                 start=True, stop=True)
                sig = sig_pool.tile([128, 512], BF16, tag="sig")
                nc.scalar.activation(sig, lg, AF.Sigmoid, bias=cbs_sb[:, 0:1],
                                     scale=gate["a"][:, 0:1])
                last_sig = sig
                make_srow(p, sig)
                lag = LAG if p < 104 else max(1, LAG - (p - 104))
                drive_conv(p - lag)
            return last_sig

        BLOCKS = [24, 40]
        starts = [sum(BLOCKS[:i]) for i in range(len(BLOCKS))]
        for g in range(BLOCKS[0]):
            prefetch(g)
        fetched = BLOCKS[0]
        for bi, bsz in enumerate(BLOCKS):
            g0 = starts[bi]
            last_r = None
            for g in range(g0, g0 + bsz):
                last_r = phase_a(g)
            gate["a"] = make_gate(last_r[:, 0:1])
            last_sig = None
            nxt = BLOCKS[bi + 1] if bi + 1 < len(BLOCKS) else 0
            for j, g in enumerate(range(g0, g0 + bsz)):
                # prefetch the next block's groups spread over this B phase
                while fetched < min(NG, g0 + bsz + nxt) and \
                        (j + 1) * nxt >= (fetched - g0 - bsz + 1) * bsz:
                    prefetch(fetched)
                    fetched += 1
                last_sig = phase_b(g)
            gate["b"] = make_gate(last_sig[:, 0:1])
        drive_conv(127)
        pscv_ctx.__exit__(None, None, None)
        pslg_ctx.__exit__(None, None, None)

    nc.finalize()
    return nc


def _fold_weights(qW, qb, vW, vb, K, qn_g, qn_b, vn_g, vn_b, cW, cb):
    f = np.float32
    qW, qb, vW, vb, K = f(qW), f(qb), f(vW), f(vb), f(K)
    qn_g, qn_b, vn_g, vn_b, cW, cb = f(qn_g), f(qn_b), f(vn_g), f(vn_b), f(cW), f(cb)
    scale = np.float32(64.0 ** -0.5)
    qWf = qn_g[:, None] * qW.T                      # [64k, 64co]
    bprime = qb + qW @ qn_b
    A0 = scale * (qWf @ K.T)                        # [64, 128]
    cbs = scale * (K @ bprime)                      # [128]
    At = 8.0 * (A0 - A0.mean(axis=0, keepdims=True))
    # K=128 zero-padded lhsT blocks: block0 contracts rows 0-63 (row y of the
    # pair), block1 rows 64-127 (row y+1).  Avoids K=64 matmuls at partition
    # base 64, which crash at runtime (bad tile_position (64,0)).
    Abig = np.zeros((128, 256), np.float32)
    Abig[0:64, 0:128] = At
    Abig[64:128, 128:256] = At

    bexp = np.zeros((128, 128), np.float32)
    bexp[0:64, 0:64] = 1.0
    bexp[64:128, 64:128] = 1.0

    vWf = vn_g[:, None] * vW.T                      # [128, 128]
    vbp = vb + vW @ vn_b
    vN = np.float32(np.sqrt(128.0)) * (vWf - vWf.mean(axis=0, keepdims=True))

    cw8 = np.zeros((128, 12, 2, 64), np.float32)
    for h in (0, 1):
        for dx in (0, 1, 2):
            for e in (0, 1):
                blk = h * 6 + dx * 2 + e
                if h == 0 and e == 0:
                    cw8[:, blk, 0, :] = cW[:, :, 1, dx].T
                    cw8[:, blk, 1, :] = cW[:, :, 2, dx].T
                elif h == 0 and e == 1:
                    cw8[:, blk, 1, :] = cW[:, :, 0, dx].T
                elif h == 1 and e == 0:
                    cw8[:, blk, 0, :] = cW[:, :, 0, dx].T
                    cw8[:, blk, 1, :] = cW[:, :, 1, dx].T
                else:
                    cw8[:, blk, 0, :] = cW[:, :, 2, dx].T

    onesf = np.ones((128, 128), np.float32)
    # conv bias K=1 matmul lhsT [1, 64]
    cb8 = np.zeros((1, 2, 64), np.float32)
    cb8[0, 0, :] = cb

    return {
        "Abig": np.ascontiguousarray(Abig.astype(NPBF16)),
        "bexp": np.ascontiguousarray(bexp.astype(NPBF16)),
        "cw8": np.ascontiguousarray(cw8.reshape(128, 1536).astype(NPFP8)),
        "cbs": np.ascontiguousarray(cbs.reshape(128, 1)),
        "vN": np.ascontiguousarray(vN.astype(NPBF16)),
        "vbp": np.ascontiguousarray(vbp.reshape(128, 1)),
        "onesf": np.ascontiguousarray(onesf.astype(NPBF16)),
        "cb8": np.ascontiguousarray(cb8.reshape(1, 128).astype(NPFP8)),
    }


def _pack_q(qi):
    # q [64, 256, 256] -> [128, 32768]: (c, 2j+r, x) -> [r*64+c, j*256+x]
    return np.ascontiguousarray(
        np.float32(qi).reshape(64, 128, 2, 256).transpose(2, 0, 1, 3)
        .reshape(128, 32768).astype(NPBF16))


def _unpack_out(o2):
    # conv result is already row-major [64ch, 256*256]
    return np.asarray(o2, np.float32).reshape(64, 256, 256)


def _run(in_maps, **kw):
    if "nc" not in _CACHE:
        _CACHE["nc"] = _build_nc()
    return run_bass_kernel_spmd(_CACHE["nc"], in_maps, list(range(8)), **kw)


def kernel(q, v, qW, qb, vW, vb, K, qn_g, qn_b, vn_g, vn_b, cW, cb):
    base = _fold_weights(qW, qb, vW, vb, K, qn_g, qn_b, vn_g, vn_b, cW, cb)
    in_maps = []
    for i in range(8):
        m = dict(base)
        m["q2"] = _pack_q(q[i])
        m["v2"] = np.ascontiguousarray(np.float32(v[i]).reshape(128, 4096).astype(NPBF16))
        in_maps.append(m)
    res = _run(in_maps)
    qcb = np.float32(q) + np.float32(cb)[None, :, None, None]
    outs = [qcb[i] + _unpack_out(res.results[i]["out2"]) for i in range(8)]
    return np.stack(outs)
